# revision 28
# baseline (speedup 1.0000x reference)
"""Trainium2 Bass kernel for nn_AttentionMoE: noisy top-2 MoE over 8 attention experts.

Expert-parallel over 8 NeuronCores; host does the tiny gating/loss math and routing.

Default path (sparse + token-stationary layout, bf16 weights):
  - Core e processes only its top-2-routed tokens (capacity 256, zero-gate padding).
  - mm1: lhsT = x^T token tiles (stationary, each reused across 4 moving W1
    slices of N=512) -> h token-major; tanh on ACT.
  - PE-transposes h -> h^T (32 tiles) for mm2's stationary operand.
  - mm2: exp(scores) token-major; softmax on the free dim (reduce_sum +
    reciprocal + per-partition tensor_scalar).
  - z = gate * exp(attn * x); per-core partial z [tok, D] outputs.
  - Host: scatter-add partials by routing index, log() -> y. Loss from gating.
Weight streaming: two 4MB half-matrix slabs per matmul, triple-buffered so W2
prefetches during mm1; per-token-tile epilogues overlap the next tile's matmuls.

Env switches: MOE_LAYOUT=b (weight-stationary fallback, supports biases/dense),
MOE_WDT=f32r|f32 (higher precision), MOE_DENSE=1, MOE_USE_RS=1 (ReduceScatter
combine -- collectives wedge the axon environment used for development).
"""

import os
import sys
import math

for _p in ("/opt/trn_rl_repo", "/root/.axon_site/_ro/trn_rl_repo"):
    if os.path.isdir(_p) and _p not in sys.path:
        sys.path.insert(0, _p)

import numpy as np

B, D, E, TOPK = 512, 2048, 8, 2
NOISE_EPS = 0.01
LOSS_COEF = 1e-2
NCORES = 8
P = 128
KT = D // P          # 16 contraction tiles
NT = D // P          # 16 output-row tiles
GRP = 4              # psum tiles per accumulation group
NCHUNK = 4           # ReduceScatter chunks over the dout axis
SHARD = D // NCHUNK // NCORES  # 64 rows per (chunk, rank)

USE_RS = os.environ.get("MOE_USE_RS", "") != ""       # collectives hang under axon today
LAYOUT = os.environ.get("MOE_LAYOUT", "a")            # a: token-stationary | b: weight-stationary
WDT = os.environ.get("MOE_WDT", "bf16")               # bf16 | f32r | f32
SPARSE = os.environ.get("MOE_DENSE", "") == ""        # route only top-2 tokens per expert
TOK = 256 if SPARSE else B                            # moving free dim (token capacity)
if SPARSE:
    USE_RS = False   # sparse cores have disjoint token sets; combine on host

LAST_EXEC_NS = None
LAST_PROFILE = None

_cache = {}


def _ndtr(z):
    try:
        from scipy.special import ndtr
        return ndtr(z).astype(np.float32)
    except Exception:
        from math import erf
        flat = z.ravel().astype(np.float64)
        out = np.array([0.5 * (1.0 + erf(v / math.sqrt(2.0))) for v in flat],
                       dtype=np.float32)
        return out.reshape(z.shape)


def _host_gating(x, noise, w_gate, w_noise):
    """Mirror of the reference noisy-top-k gating + loss, in numpy f32."""
    x = x.astype(np.float32, copy=False)
    clean = x @ w_gate                                   # [B, E]
    raw = x @ w_noise
    noise_std = np.logaddexp(raw, np.float32(0.0)).astype(np.float32) + np.float32(NOISE_EPS)
    logits = clean + noise * noise_std
    order = np.argsort(-logits, axis=1, kind="stable")   # descending, jax tie order
    top_idx = order[:, :TOPK + 1]
    top_vals = np.take_along_axis(logits, top_idx, axis=1)
    tv = top_vals[:, :TOPK]
    m = tv.max(axis=1, keepdims=True)
    ex = np.exp(tv - m)
    top_gates = ex / ex.sum(axis=1, keepdims=True)       # [B, K]
    gates = np.zeros_like(logits)
    np.put_along_axis(gates, top_idx[:, :TOPK], top_gates, axis=1)
    # prob-in-top-k load
    thr_in = top_vals[:, TOPK:TOPK + 1]
    thr_out = top_vals[:, TOPK - 1:TOPK]
    is_in = logits > thr_in
    prob_in = _ndtr((clean - thr_in) / noise_std)
    prob_out = _ndtr((clean - thr_out) / noise_std)
    prob = np.where(is_in, prob_in, prob_out)
    load = prob.sum(0)
    importance = gates.sum(0)

    def cv_sq(v):
        v = v.astype(np.float32)
        return np.var(v, ddof=1) / (np.mean(v) ** 2 + np.float32(1e-10))

    loss = (cv_sq(importance) + cv_sq(load)) * np.float32(LOSS_COEF)
    return gates.astype(np.float32), np.float32(loss)


def _build_program(tok=TOK):
    import concourse.bacc as bacc
    import concourse.mybir as mybir
    import concourse.tile as tile

    TOK = tok  # shadow module default; all shapes below use this capacity
    f32 = mybir.dt.float32
    bf16 = mybir.dt.bfloat16
    AF = mybir.ActivationFunctionType

    wdt = {"bf16": bf16, "f32r": mybir.dt.float32r, "f32": f32}[WDT]
    # dtype of the x^T copy used as the matmul moving operand / of hT
    two_x = (WDT == "bf16")   # separate bf16 matmul copy of x^T next to the f32 one

    nc = bacc.Bacc("TRN2", target_bir_lowering=False, debug=False, num_devices=NCORES)

    xdt = wdt if WDT == "f32r" else f32
    xt = nc.dram_tensor("xt", [P, KT, TOK], xdt, kind="ExternalInput")
    if two_x:
        xtm = nc.dram_tensor("xtm", [P, KT, TOK], wdt, kind="ExternalInput")
    w1 = nc.dram_tensor("w1", [P, KT, D], wdt, kind="ExternalInput")
    w2 = nc.dram_tensor("w2", [P, KT, D], wdt, kind="ExternalInput")
    b1 = nc.dram_tensor("b1", [P, KT], f32, kind="ExternalInput")
    b2 = nc.dram_tensor("b2", [P, KT], f32, kind="ExternalInput")
    gate = nc.dram_tensor("gate", [TOK], f32, kind="ExternalInput")
    if USE_RS:
        yt = nc.dram_tensor("yt", [NCHUNK, SHARD, TOK], f32, kind="ExternalOutput")
    else:
        zt_out = nc.dram_tensor("zt", [D, TOK], f32, kind="ExternalOutput")

    with tile.TileContext(nc) as tc:
        with tc.tile_pool(name="singles", bufs=1) as singles, \
             tc.tile_pool(name="wslab", bufs=2) as wpool, \
             tc.tile_pool(name="zpool", bufs=3) as zpool, \
             tc.tile_pool(name="lpool", bufs=2) as lpool, \
             tc.tile_pool(name="psum_mm", bufs=7, space="PSUM") as psum_mm, \
             tc.tile_pool(name="psum_misc", bufs=1, space="PSUM") as psum_misc, \
             tc.tile_pool(name="dram", bufs=1, space="DRAM") as dram:

            # ---- resident inputs (host supplies partition-major layouts) ----
            xtf = singles.tile([P, KT, TOK], xdt)
            nc.sync.dma_start(out=xtf[:], in_=xt[:])
            if two_x:
                xtw = singles.tile([P, KT, TOK], wdt)
                nc.sync.dma_start(out=xtw[:], in_=xtm[:])
            else:
                xtw = xtf

            b1sb = singles.tile([P, KT], f32)
            nc.sync.dma_start(out=b1sb[:], in_=b1[:])
            b2sb = singles.tile([P, NT], f32)
            nc.sync.dma_start(out=b2sb[:], in_=b2[:])

            # gate broadcast [P, TOK] via partition-stride-0 DMA
            gateB = singles.tile([P, TOK], f32)
            nc.sync.dma_start(out=gateB[:],
                              in_=gate[:].rearrange("(o t) -> o t", o=1)
                                  .to_broadcast([P, TOK]))

            ones_sum = singles.tile([P, P], bf16)
            nc.vector.memset(ones_sum[:], 1.0)

            hT = singles.tile([P, KT, TOK], wdt)
            expT = singles.tile([P, NT, TOK], f32)
            expB = singles.tile([P, NT, TOK], bf16)


            HALF = NT // 2   # dout tiles per weight slab (one big DMA each)

            def mm_pass(w, src3d, consume):
                for half in range(2):
                    slab = wpool.tile([P, KT, HALF * P], wdt, tag="wslab",
                                      name=f"wslab{half}")
                    nc.sync.dma_start(
                        out=slab[:],
                        in_=w[:, :, half * HALF * P:(half + 1) * HALF * P])
                    for gl in range(HALF // GRP):
                        g = half * (HALF // GRP) + gl
                        ptiles = [psum_mm.tile([P, TOK], f32, tag="mm",
                                               name=f"mmps{g}_{j}")
                                  for j in range(GRP)]
                        for kt in range(KT):
                            for j in range(GRP):
                                c0 = (gl * GRP + j) * P
                                nc.tensor.matmul(
                                    ptiles[j][:],
                                    slab[:, kt, c0:c0 + P],
                                    src3d[:, kt, :],
                                    start=(kt == 0), stop=(kt == KT - 1))
                        for j in range(GRP):
                            consume(g * GRP + j, ptiles[j])

            # ---- mm1: hT = tanh(x @ W1 + b1)^T ----
            def consume1(t, pt):
                nc.scalar.activation(hT[:, t, :], pt[:], AF.Tanh,
                                     bias=b1sb[:, t:t + 1])
            mm_pass(w1, xtw if two_x else xtf, consume1)

            # ---- mm2: expT = exp((hT . W2) + b2) ----
            def consume2(t, pt):
                nc.scalar.activation(expT[:, t, :], pt[:], AF.Exp,
                                     bias=b2sb[:, t:t + 1])
                nc.vector.tensor_copy(expB[:, t, :], expT[:, t, :])
            mm_pass(w2, hT, consume2)

            # ---- sumexp via PE all-ones matmul (consecutive accumulation) ----
            se_ps = psum_misc.tile([P, TOK], f32, tag="misc")
            for t in range(NT):
                nc.tensor.matmul(se_ps[:], ones_sum[:], expB[:, t, :],
                                 start=(t == 0), stop=(t == NT - 1))

            # ---- softmax denominator (already broadcast across partitions) ----
            invB = singles.tile([P, TOK], f32)
            nc.vector.reciprocal(invB[:], se_ps[:])

            # ---- z^T tiles: gate * exp(attn * x) ----
            if USE_RS:
                zts = [dram.tile([D // NCHUNK, TOK], f32, tag=f"zt{c}", name=f"ztc{c}")
                       for c in range(NCHUNK)]
                rs_outs = [dram.tile([SHARD, TOK], f32, tag=f"rs{c}", name=f"rsc{c}")
                           for c in range(NCHUNK)]

            per_chunk = NT // NCHUNK
            for t in range(NT):
                t1 = zpool.tile([P, TOK], f32, tag="t1")
                nc.vector.tensor_mul(t1[:], expT[:, t, :], invB[:])
                t2 = zpool.tile([P, TOK], f32, tag="t2")
                xv = xtf[:, t, :]
                if WDT == "f32r":
                    xv = xv.bitcast(f32)
                nc.vector.tensor_mul(t2[:], t1[:], xv)
                z = zpool.tile([P, TOK], f32, tag="z")
                nc.scalar.activation(z[:], t2[:], AF.Exp)
                zg = zpool.tile([P, TOK], f32, tag="zg")
                nc.vector.tensor_mul(zg[:], z[:], gateB[:])
                if USE_RS:
                    c, r = divmod(t, per_chunk)
                    nc.sync.dma_start(out=zts[c][r * P:(r + 1) * P, :], in_=zg[:])
                else:
                    nc.sync.dma_start(out=zt_out[t * P:(t + 1) * P, :], in_=zg[:])

            # ---- combine: ReduceScatter over dout axis, then log ----
            if USE_RS:
                import concourse.mybir as mybir2
                for c in range(NCHUNK):
                    nc.gpsimd.collective_compute(
                        "ReduceScatter",
                        mybir2.AluOpType.add,
                        replica_groups=[list(range(NCORES))],
                        ins=[zts[c].opt()],
                        outs=[rs_outs[c].opt()],
                    )
                for c in range(NCHUNK):
                    lt = lpool.tile([SHARD, TOK], f32, tag="lt")
                    nc.sync.dma_start(out=lt[:], in_=rs_outs[c][:])
                    lo = lpool.tile([SHARD, TOK], f32, tag="lo")
                    nc.scalar.activation(lo[:], lt[:], AF.Ln)
                    nc.sync.dma_start(out=yt[c], in_=lo[:])

    nc.compile()
    return nc


def _round_f32r(a):
    u = np.ascontiguousarray(a, dtype=np.float32).view(np.uint32)
    lsb = (u >> np.uint32(12)) & np.uint32(1)
    r = (u + np.uint32(0x7FF) + lsb) & np.uint32(0xFFFFF000)
    return r.view(np.float32)


def _build_program_a(tok=TOK):
    """Token-stationary layout: lhsT = x^T / h^T tiles (reused across 4 moving
    W slices per load), outputs token-major, softmax on the free dim.
    Requires zero biases (the reference always has b1=b2=0)."""
    import concourse.bacc as bacc
    import concourse.mybir as mybir
    import concourse.tile as tile

    TOK = tok
    MT = TOK // P
    f32 = mybir.dt.float32
    bf16 = mybir.dt.bfloat16
    AF = mybir.ActivationFunctionType
    wdt = {"bf16": bf16, "f32r": mybir.dt.float32r, "f32": f32}[WDT]

    nc = bacc.Bacc("TRN2", target_bir_lowering=False, debug=False, num_devices=NCORES)

    xtm = nc.dram_tensor("xtm", [P, KT, TOK], wdt, kind="ExternalInput")
    xg = nc.dram_tensor("xg", [P, MT, D], f32, kind="ExternalInput")
    w1 = nc.dram_tensor("w1", [P, KT, D], wdt, kind="ExternalInput")
    w2 = nc.dram_tensor("w2", [P, KT, D], wdt, kind="ExternalInput")
    gatec = nc.dram_tensor("gatec", [P, MT], f32, kind="ExternalInput")
    ident = nc.dram_tensor("ident", [P, P], wdt, kind="ExternalInput")
    zt_out = nc.dram_tensor("zt", [TOK, D], f32, kind="ExternalOutput")

    HALF2 = D // 2   # moving-dim columns per weight slab

    with tile.TileContext(nc) as tc:
        with tc.tile_pool(name="singles", bufs=1) as singles, \
             tc.tile_pool(name="wslab", bufs=3) as wpool, \
             tc.tile_pool(name="zpool", bufs=2) as zpool, \
             tc.tile_pool(name="psum_mm", bufs=6, space="PSUM") as psum_mm, \
             tc.tile_pool(name="psum_tr", bufs=2, space="PSUM") as psum_tr:

            xtw = singles.tile([P, KT, TOK], wdt)
            nc.sync.dma_start(out=xtw[:], in_=xtm[:])
            xgs = singles.tile([P, MT, D], f32)
            nc.sync.dma_start(out=xgs[:], in_=xg[:])
            gcol = singles.tile([P, MT], f32)
            nc.sync.dma_start(out=gcol[:], in_=gatec[:])
            idn = singles.tile([P, P], wdt)
            nc.sync.dma_start(out=idn[:], in_=ident[:])

            hsb = singles.tile([P, MT, D], wdt)     # h, token-major
            hT = singles.tile([P, KT, TOK], wdt)    # h transposed for mm2
            essb = singles.tile([P, MT, D], f32)    # exp(scores)

            def mm_pass(w, src3d, consume):
                slabs = []
                for half in range(2):
                    slab = wpool.tile([P, KT, HALF2], wdt, tag="wslab",
                                      name=f"awslab{half}")
                    nc.sync.dma_start(
                        out=slab[:],
                        in_=w[:, :, half * HALF2:(half + 1) * HALF2])
                    slabs.append(slab)
                for mt in range(MT):
                    for half in range(2):
                        ptiles = [psum_mm.tile([P, 512], f32, tag="mm",
                                               name=f"aps{half}_{mt}_{j}")
                                  for j in range(2)]
                        for kt in range(KT):
                            for j in range(2):
                                nc.tensor.matmul(
                                    ptiles[j][:],
                                    src3d[:, kt, mt * P:(mt + 1) * P],
                                    slabs[half][:, kt, j * 512:(j + 1) * 512],
                                    start=(kt == 0), stop=(kt == KT - 1))
                        for j in range(2):
                            consume(mt, half * 2 + j, ptiles[j])

            # ---- mm1: h = tanh(x @ W1), token-major ----
            def consume1(mt, nt, pt):
                nc.scalar.activation(hsb[:, mt, nt * 512:(nt + 1) * 512],
                                     pt[:], AF.Tanh)
            mm_pass(w1, xtw, consume1)

            # ---- transpose h -> hT for mm2's stationary ----
            for mt in range(MT):
                for c in range(KT):
                    tp = psum_tr.tile([P, P], wdt, tag="tr", name=f"tr{mt}_{c}")
                    nc.tensor.transpose(tp[:], hsb[:, mt, c * P:(c + 1) * P],
                                        idn[:])
                    nc.vector.tensor_copy(hT[:, c, mt * P:(mt + 1) * P], tp[:])

            # ---- mm2: exp(scores), token-major ----
            def consume2(mt, nt, pt):
                nc.scalar.activation(essb[:, mt, nt * 512:(nt + 1) * 512],
                                     pt[:], AF.Exp)
            mm_pass(w2, hT, consume2)

            # ---- free-dim softmax fused into one ACT:
            #      gate*exp(attn*x) = exp(inv*(es*x) + ln(gate)) ----
            for mt in range(MT):
                se = zpool.tile([P, 1], f32, tag="se")
                nc.vector.reduce_sum(se[:], essb[:, mt, :],
                                     axis=bass_axis_x())
                inv = zpool.tile([P, 1], f32, tag="inv")
                nc.vector.reciprocal(inv[:], se[:])
                t2 = zpool.tile([P, D], f32, tag="tmp", name=f"t2_{mt}")
                nc.vector.tensor_mul(t2[:], essb[:, mt, :], xgs[:, mt, :])
                zg = zpool.tile([P, D], f32, tag="zz", name=f"zg_{mt}")
                nc.scalar.activation(zg[:], t2[:], AF.Exp,
                                     bias=gcol[:, mt:mt + 1], scale=inv[:])
                nc.sync.dma_start(out=zt_out[mt * P:(mt + 1) * P, :], in_=zg[:])

    nc.compile()
    return nc


def bass_axis_x():
    import concourse.mybir as mybir
    return mybir.AxisListType.X


def _to_wdt_np(a):
    if WDT == "bf16":
        import ml_dtypes
        return a.astype(ml_dtypes.bfloat16)
    if WDT == "f32r":
        return _round_f32r(a)
    return np.asarray(a, dtype=np.float32)


def kernel(x, noise, w_gate, w_noise, W1, b1, W2, b2):
    global LAST_EXEC_NS, LAST_PROFILE
    from concourse.bass_utils import run_bass_kernel_spmd

    x = np.asarray(x, dtype=np.float32)
    noise = np.asarray(noise, dtype=np.float32)
    w_gate = np.asarray(w_gate, dtype=np.float32)
    w_noise = np.asarray(w_noise, dtype=np.float32)
    W1 = np.asarray(W1, dtype=np.float32)
    b1 = np.asarray(b1, dtype=np.float32)
    W2 = np.asarray(W2, dtype=np.float32)
    b2 = np.asarray(b2, dtype=np.float32)

    gates, loss = _host_gating(x, noise, w_gate, w_noise)

    tok = TOK
    if SPARSE:
        need = int(np.bincount(np.nonzero(gates > 0)[1], minlength=NCORES).max())
        tok = max(TOK, -(-need // P) * P)
    use_a = (LAYOUT == "a" and SPARSE and not USE_RS
             and not (np.any(b1) or np.any(b2)))
    key = (tok, use_a)
    if _cache.get("key") != key:
        _cache["prog"] = _build_program_a(tok) if use_a else _build_program(tok)
        _cache["key"] = key
    nc = _cache["prog"]

    xt = np.ascontiguousarray(x.T)

    def shuf_kp(a):
        # [D, N] -> [128, KT, N] with D = KT*128 split as (k p)
        return np.ascontiguousarray(a.reshape(KT, P, a.shape[1]).transpose(1, 0, 2))

    idxs = []
    in_maps = []
    for e in range(NCORES):
        if SPARSE:
            idx = np.nonzero(gates[:, e] > 0)[0]
            assert len(idx) <= tok, f"expert {e} overflow: {len(idx)} > {tok}"
            pad = np.zeros(tok - len(idx), dtype=idx.dtype)
            idx_pad = np.concatenate([idx, pad])
            g = np.zeros(tok, dtype=np.float32)
            g[:len(idx)] = gates[idx, e]
            xte = np.ascontiguousarray(xt[:, idx_pad])
        else:
            idx = None
            g = np.ascontiguousarray(gates[:, e])
            xte = xt
        if WDT == "f32r":
            xte = _round_f32r(xte)
        idxs.append(idx)
        if use_a:
            mt = tok // P
            m = {
                "xtm": shuf_kp(_to_wdt_np(xte)),
                "xg": np.ascontiguousarray(
                    xte.T.reshape(mt, P, D).transpose(1, 0, 2)),
                "w1": shuf_kp(_to_wdt_np(W1[e])),
                "w2": shuf_kp(_to_wdt_np(W2[e])),
                "gatec": np.ascontiguousarray(
                    np.log(np.maximum(g, 1e-38)).astype(np.float32)
                    .reshape(mt, P).T),
                "ident": _to_wdt_np(np.eye(P, dtype=np.float32)),
            }
        else:
            m = {
                "xt": shuf_kp(xte),
                "w1": shuf_kp(_to_wdt_np(W1[e])),
                "w2": shuf_kp(_to_wdt_np(W2[e])),
                "b1": np.ascontiguousarray(b1[e].reshape(KT, P).T),
                "b2": np.ascontiguousarray(b2[e].reshape(KT, P).T),
                "gate": g,
            }
            if WDT == "bf16":
                m["xtm"] = shuf_kp(_to_wdt_np(xte))
        in_maps.append(m)

    trace = os.environ.get("MOE_TRACE", "") != ""
    kw = {}
    if trace:
        import tempfile
        kw["trace"] = True
        kw["tmpdir"] = os.environ.get("MOE_TRACE_DIR") or tempfile.mkdtemp(
            prefix="moe_trace_")
    br = run_bass_kernel_spmd(nc, in_maps, list(range(NCORES)), **kw)
    res = br.results
    LAST_EXEC_NS = br.exec_time_ns
    LAST_PROFILE = br.profile_json
    if trace and br.instructions_and_trace is not None:
        _cache["trace_path"] = br.instructions_and_trace[1]

    if USE_RS:
        yT = np.empty((D, TOK), dtype=np.float32)
        for r in range(NCORES):
            ytr = res[r]["yt"]  # [NCHUNK, SHARD, TOK]
            for c in range(NCHUNK):
                base = c * (D // NCHUNK) + r * SHARD
                yT[base:base + SHARD] = ytr[c]
        y = np.ascontiguousarray(yT.T)
    elif SPARSE and use_a:
        ztot = np.zeros((B, D), dtype=np.float64)
        for r in range(NCORES):
            n = len(idxs[r])
            ztot[idxs[r], :] += res[r]["zt"][:n, :]
        y = np.ascontiguousarray(np.log(ztot).astype(np.float32))
    elif SPARSE:
        ztot = np.zeros((D, B), dtype=np.float64)
        for r in range(NCORES):
            n = len(idxs[r])
            ztot[:, idxs[r]] += res[r]["zt"][:, :n]
        y = np.ascontiguousarray(np.log(ztot).T.astype(np.float32))
    else:
        ztot = np.zeros((D, tok), dtype=np.float64)
        for r in range(NCORES):
            ztot += res[r]["zt"]
        y = np.ascontiguousarray(np.log(ztot).T.astype(np.float32))

    return y, loss
